# revision 10
# baseline (speedup 1.0000x reference)
"""Local (windowed) attention Trainium2 Bass kernel.

Problem: q,k,v [8, 8, 4096, 64] fp32; window 128, look_backward 1, pad -1.0.
out[b,h,w,i,:] = softmax(scale * q_wi . [k_{w-1}; k_w]) @ [v_{w-1}; v_w]
(with window -1 = all -1.0 pad values, which DO enter the softmax).

Sharding: data-parallel over flat batch*heads (64) -> 8 heads per core.

Per-core layouts (prepared host-side):
  qT : [4, 128, 4096]  float32r  - head pair stacked on partitions (d=64 each),
                                   free axis = 4096 queries (d-major transposed)
  kT : [4, 128, 4224]  float32r  - same, with one pad chunk (128 keys of -1.0)
                                   prepended -> 33 chunks of 128 keys
  v  : [8, 128, 33, 65] float16  - per head; partition = key-within-chunk,
                                   pad chunk prepended; col 64 = 1.0 (ones
                                   column yields softmax denominator l)
  out: [8, 128, 32, 64] float32  - partition = query-within-window

Device pipeline per head pair, per key chunk p (0..32):
  MM1 (fp32r): scoresT[j, i] for the <=2 windows attending chunk p
               lhsT = kT[:, p-chunk] [64,128], rhs = qT slice [64,<=256]
  ACT exp (scale=1/8) psum -> fp16 P tiles (batched over 2 chunks x 2 heads)
  MM2 (fp16): out_w[i, 0:65] += P_blockT @ v_aug[p]   (col 64 accumulates l)
  DVE: evacuate completed window psum -> sbuf; per head: reciprocal(l),
       broadcast-multiply, one contiguous DMA store.
"""

import os
import sys

for _p in ("/opt/trn_rl_repo", "/opt/pypackages"):
    if os.path.isdir(_p) and _p not in sys.path:
        sys.path.append(_p)

import numpy as np

import concourse.mybir as mybir
import concourse.tile as tile
from concourse import bacc
from concourse.bass_utils import run_bass_kernel_spmd

B, H, N, D = 8, 8, 4096, 64
WS = 128                 # window size
W = N // WS              # 32 windows
C = W + 1                # 33 key chunks incl. pad chunk
NC = 8                   # cores
HPC = (B * H) // NC      # 8 heads per core
PAIRS = HPC // 2         # 4 head pairs per core
SCALE = float(D) ** -0.5

MM1_DT = mybir.dt.float16
MM2_DT = mybir.dt.float16
GROUP = 2                # key chunks per exp batch

_NC_CACHE = {}


def build_nc(pairs=PAIRS, w=W):
    c = w + 1
    n = w * WS
    nc = bacc.Bacc("TRN2", target_bir_lowering=False)
    qT = nc.dram_tensor("qT", [pairs, 128, n], MM1_DT, kind="ExternalInput")
    kT = nc.dram_tensor("kT", [pairs, 128, c * WS], MM1_DT, kind="ExternalInput")
    vv = nc.dram_tensor("v", [2 * pairs, 128, c, D + 1], MM2_DT, kind="ExternalInput")
    out = nc.dram_tensor("out", [2 * pairs, 128, w, D], mybir.dt.float32,
                         kind="ExternalOutput")

    f32 = mybir.dt.float32
    Exp = mybir.ActivationFunctionType.Exp

    with tile.TileContext(nc) as tc:
        with (
            tc.tile_pool(name="qk", bufs=2) as qk_pool,
            tc.tile_pool(name="vp", bufs=4) as v_pool,
            tc.tile_pool(name="pt", bufs=3) as pt_pool,
            tc.tile_pool(name="un", bufs=4) as un_pool,
            tc.tile_pool(name="st", bufs=2) as st_pool,
            tc.tile_pool(name="rc", bufs=2) as rc_pool,
            tc.tile_pool(name="ps_s", bufs=2, space="PSUM") as ps_s,
            tc.tile_pool(name="ps_o", bufs=4, space="PSUM") as ps_o,
        ):
            # PE warm-up: ~5us of dummy back-to-back matmuls during the
            # initial DMA fill so HAM flips to 8/8 before real work. Gaps in
            # the real stream stay well under the ~3.4us MID window, so the
            # clock stays warm for the whole kernel.
            wu_w = qk_pool.tile([64, 128], MM2_DT, tag="warm_w", name="wu_w")
            nc.vector.memset(wu_w[:], 0.0)
            wu_ps = ps_s.tile([128, GROUP * 2 * 256], f32, tag="scores",
                              name="wu_ps")
            for _ in range(44):
                nc.tensor.matmul(wu_ps[:, 0:128], wu_w[:], wu_w[:],
                                 start=True, stop=True)
            for pair in range(pairs):
                qt = qk_pool.tile([128, n], MM1_DT, tag="qT")
                kt = qk_pool.tile([128, c * WS], MM1_DT, tag="kT")
                nc.sync.dma_start(qt[:], qT[pair])
                nc.sync.dma_start(kt[:], kT[pair])
                vts = []
                for h in range(2):
                    vt = v_pool.tile([128, c, D + 1], MM2_DT, tag="v")
                    nc.sync.dma_start(vt[:], vv[2 * pair + h])
                    vts.append(vt)

                unnorm = [un_pool.tile([128, w, D + 1], f32, tag="un",
                                       name=f"un_{pair}_{h}")
                          for h in range(2)]
                out_ps = {}  # (h, w) -> psum tile

                groups = [list(range(g, min(g + GROUP, c)))
                          for g in range(0, c, GROUP)]
                for chunks in groups:
                    ps = ps_s.tile([128, GROUP * 2 * 256], f32, tag="scores")
                    # MM1s
                    runs = []  # written (col, n) regions
                    for s, p in enumerate(chunks):
                        qlo = max(0, (p - 1) * WS)
                        qhi = min(n, (p + 1) * WS)
                        if p == 0:
                            qhi = min(n, 2 * WS)  # avoid garbage: fill 256
                        nq = qhi - qlo
                        for h in range(2):
                            col = h * (GROUP * 256) + s * 256
                            nc.tensor.matmul(
                                ps[:, col:col + nq],
                                kt[64 * h:64 * h + 64, p * WS:(p + 1) * WS],
                                qt[64 * h:64 * h + 64, qlo:qhi],
                                start=True, stop=True,
                            )
                            runs.append((col, nq))
                    # batched exp: merge adjacent written runs
                    pt = pt_pool.tile([128, GROUP * 2 * 256], MM2_DT, tag="pt")
                    merged = []
                    for rcol, rn in sorted(runs):
                        if merged and merged[-1][0] + merged[-1][1] == rcol:
                            merged[-1][1] += rn
                        else:
                            merged.append([rcol, rn])
                    for rcol, rn in merged:
                        nc.scalar.activation(pt[:, rcol:rcol + rn],
                                             ps[:, rcol:rcol + rn],
                                             Exp, scale=SCALE)
                    # MM2s + evacuation of completed windows
                    for s, p in enumerate(chunks):
                        for h in range(2):
                            col = h * (GROUP * 256) + s * 256
                            if p >= 1:
                                # block 0: window p-1 self-contribution (last)
                                wi = p - 1
                                t = out_ps[(h, wi)]
                                nc.tensor.matmul(
                                    t[:, 0:D + 1],
                                    pt[:, col:col + WS],
                                    vts[h][:, p, :],
                                    start=False, stop=True,
                                )
                                nc.vector.tensor_copy(unnorm[h][:, wi, :],
                                                      t[:, 0:D + 1])
                                del out_ps[(h, wi)]
                            if p <= w - 1:
                                # window p prev-contribution (first)
                                bcol = col + (WS if p >= 1 else 0)
                                t = ps_o.tile([128, D + 1], f32, tag="out")
                                out_ps[(h, p)] = t
                                nc.tensor.matmul(
                                    t[:, 0:D + 1],
                                    pt[:, bcol:bcol + WS],
                                    vts[h][:, p, :],
                                    start=True, stop=False,
                                )
                # per-head epilogue
                for h in range(2):
                    recip = rc_pool.tile([128, w], f32, tag="recip")
                    nc.vector.reciprocal(recip[:], unnorm[h][:, :, D])
                    stg = st_pool.tile([128, w, D], f32, tag="stg")
                    nc.vector.tensor_mul(
                        stg[:],
                        unnorm[h][:, :, 0:D],
                        recip[:, :, None].to_broadcast((128, w, D)),
                    )
                    nc.sync.dma_start(out[2 * pair + h], stg[:])

    nc.compile()
    return nc


def _get_nc():
    if "nc" not in _NC_CACHE:
        _NC_CACHE["nc"] = build_nc()
    return _NC_CACHE["nc"]


def _prep_core(qf, kf, vf, lo):
    """Build one core's input dict from flat [64, 4096, 64] fp32 arrays."""
    q8 = qf[lo:lo + HPC]                      # [8, 4096, 64]
    k8 = kf[lo:lo + HPC]
    v8 = vf[lo:lo + HPC]

    qT = np.ascontiguousarray(q8.transpose(0, 2, 1)).reshape(PAIRS, 128, N)
    qT = qT.astype(np.float16)

    pad = np.full((HPC, WS, D), -1.0, dtype=np.float32)
    kp = np.concatenate([pad, k8], axis=1)    # [8, 4224, 64]
    kT = np.ascontiguousarray(kp.transpose(0, 2, 1)).reshape(PAIRS, 128, C * WS)
    kT = kT.astype(np.float16)

    vp = np.concatenate([pad, v8], axis=1)    # [8, 4224, 64]
    ones = np.ones((HPC, C * WS, 1), dtype=np.float32)
    va = np.concatenate([vp, ones], axis=2)   # [8, 4224, 65]
    va = va.reshape(HPC, C, WS, D + 1).transpose(0, 2, 1, 3)  # [8, 128, 33, 65]
    va = np.ascontiguousarray(va).astype(np.float16)

    return {"qT": qT, "kT": kT, "v": va}


def kernel(q, k, v):
    q = np.asarray(q, dtype=np.float32)
    k = np.asarray(k, dtype=np.float32)
    v = np.asarray(v, dtype=np.float32)
    qf = q.reshape(B * H, N, D)
    kf = k.reshape(B * H, N, D)
    vf = v.reshape(B * H, N, D)

    nc = _get_nc()
    in_maps = [_prep_core(qf, kf, vf, HPC * c) for c in range(NC)]
    res = run_bass_kernel_spmd(nc, in_maps, core_ids=list(range(NC)))

    outs = []
    for c in range(NC):
        o = res.results[c]["out"]             # [8, 128, 32, 64]
        o = o.transpose(0, 2, 1, 3).reshape(HPC, N, D)
        outs.append(o)
    return np.concatenate(outs, axis=0).reshape(B, H, N, D).astype(np.float32)


if __name__ == "__main__":
    rng = np.random.default_rng(0)
    q = rng.standard_normal((B, H, N, D), dtype=np.float32)
    k = rng.standard_normal((B, H, N, D), dtype=np.float32)
    v = rng.standard_normal((B, H, N, D), dtype=np.float32)
    o = kernel(q, k, v)
    print("out", o.shape, o.dtype, float(np.abs(o).max()))


# revision 11
# speedup vs baseline: 1.0060x; 1.0060x over previous
"""Local (windowed) attention Trainium2 Bass kernel.

Problem: q,k,v [8, 8, 4096, 64] fp32; window 128, look_backward 1, pad -1.0.
out[b,h,w,i,:] = softmax(scale * q_wi . [k_{w-1}; k_w]) @ [v_{w-1}; v_w]
(with window -1 = all -1.0 pad values, which DO enter the softmax).

Sharding: data-parallel over flat batch*heads (64) -> 8 heads per core.

Per-core layouts (prepared host-side):
  qT : [4, 128, 4096]  float32r  - head pair stacked on partitions (d=64 each),
                                   free axis = 4096 queries (d-major transposed)
  kT : [4, 128, 4224]  float32r  - same, with one pad chunk (128 keys of -1.0)
                                   prepended -> 33 chunks of 128 keys
  v  : [8, 128, 33, 65] float16  - per head; partition = key-within-chunk,
                                   pad chunk prepended; col 64 = 1.0 (ones
                                   column yields softmax denominator l)
  out: [8, 128, 32, 64] float32  - partition = query-within-window

Device pipeline per head pair, per key chunk p (0..32):
  MM1 (fp32r): scoresT[j, i] for the <=2 windows attending chunk p
               lhsT = kT[:, p-chunk] [64,128], rhs = qT slice [64,<=256]
  ACT exp (scale=1/8) psum -> fp16 P tiles (batched over 2 chunks x 2 heads)
  MM2 (fp16): out_w[i, 0:65] += P_blockT @ v_aug[p]   (col 64 accumulates l)
  DVE: evacuate completed window psum -> sbuf; per head: reciprocal(l),
       broadcast-multiply, one contiguous DMA store.
"""

import os
import sys

for _p in ("/opt/trn_rl_repo", "/opt/pypackages"):
    if os.path.isdir(_p) and _p not in sys.path:
        sys.path.append(_p)

import numpy as np

import concourse.mybir as mybir
import concourse.tile as tile
from concourse import bacc
from concourse.bass_utils import run_bass_kernel_spmd

B, H, N, D = 8, 8, 4096, 64
WS = 128                 # window size
W = N // WS              # 32 windows
C = W + 1                # 33 key chunks incl. pad chunk
NC = 8                   # cores
HPC = (B * H) // NC      # 8 heads per core
PAIRS = HPC // 2         # 4 head pairs per core
SCALE = float(D) ** -0.5

MM1_DT = mybir.dt.float16
MM2_DT = mybir.dt.float16
GROUP = 2                # key chunks per exp batch

_NC_CACHE = {}


def build_nc(pairs=PAIRS, w=W):
    c = w + 1
    n = w * WS
    nc = bacc.Bacc("TRN2", target_bir_lowering=False)
    qT = nc.dram_tensor("qT", [pairs, 128, n], MM1_DT, kind="ExternalInput")
    kT = nc.dram_tensor("kT", [pairs, 128, c * WS], MM1_DT, kind="ExternalInput")
    vv = nc.dram_tensor("v", [2 * pairs, 128, c, D + 1], MM2_DT, kind="ExternalInput")
    out = nc.dram_tensor("out", [2 * pairs, 128, w, D], mybir.dt.float32,
                         kind="ExternalOutput")

    f32 = mybir.dt.float32
    Exp = mybir.ActivationFunctionType.Exp

    with tile.TileContext(nc) as tc:
        with (
            tc.tile_pool(name="qk", bufs=2) as qk_pool,
            tc.tile_pool(name="vp", bufs=4) as v_pool,
            tc.tile_pool(name="pt", bufs=3) as pt_pool,
            tc.tile_pool(name="un", bufs=4) as un_pool,
            tc.tile_pool(name="st", bufs=2) as st_pool,
            tc.tile_pool(name="rc", bufs=2) as rc_pool,
            tc.tile_pool(name="ps_s", bufs=2, space="PSUM") as ps_s,
            tc.tile_pool(name="ps_o", bufs=4, space="PSUM") as ps_o,
        ):
            # PE warm-up: ~5us of dummy back-to-back matmuls during the
            # initial DMA fill so HAM flips to 8/8 before real work. Gaps in
            # the real stream stay well under the ~3.4us MID window, so the
            # clock stays warm for the whole kernel.
            wu_w = qk_pool.tile([64, 128], MM2_DT, tag="warm_w", name="wu_w")
            nc.vector.memset(wu_w[:], 0.0)
            wu_ps = ps_s.tile([128, GROUP * 2 * 256], f32, tag="scores",
                              name="wu_ps")
            for _ in range(100):
                nc.tensor.matmul(wu_ps[:, 0:128], wu_w[:], wu_w[:],
                                 start=True, stop=True)
            # keep the warmup live (read it) so DCE can't drop it
            wu_sink = qk_pool.tile([128, 1], f32, tag="warm_sink",
                                   name="wu_sink")
            nc.vector.tensor_copy(wu_sink[:], wu_ps[:, 0:1])
            for pair in range(pairs):
                qt = qk_pool.tile([128, n], MM1_DT, tag="qT")
                kt = qk_pool.tile([128, c * WS], MM1_DT, tag="kT")
                nc.sync.dma_start(qt[:], qT[pair])
                nc.sync.dma_start(kt[:], kT[pair])
                vts = []
                for h in range(2):
                    vt = v_pool.tile([128, c, D + 1], MM2_DT, tag="v")
                    nc.sync.dma_start(vt[:], vv[2 * pair + h])
                    vts.append(vt)

                unnorm = [un_pool.tile([128, w, D + 1], f32, tag="un",
                                       name=f"un_{pair}_{h}")
                          for h in range(2)]
                out_ps = {}  # (h, w) -> psum tile

                groups = [list(range(g, min(g + GROUP, c)))
                          for g in range(0, c, GROUP)]
                for chunks in groups:
                    ps = ps_s.tile([128, GROUP * 2 * 256], f32, tag="scores")
                    # MM1s
                    runs = []  # written (col, n) regions
                    for s, p in enumerate(chunks):
                        qlo = max(0, (p - 1) * WS)
                        qhi = min(n, (p + 1) * WS)
                        if p == 0:
                            qhi = min(n, 2 * WS)  # avoid garbage: fill 256
                        nq = qhi - qlo
                        for h in range(2):
                            col = h * (GROUP * 256) + s * 256
                            nc.tensor.matmul(
                                ps[:, col:col + nq],
                                kt[64 * h:64 * h + 64, p * WS:(p + 1) * WS],
                                qt[64 * h:64 * h + 64, qlo:qhi],
                                start=True, stop=True,
                            )
                            runs.append((col, nq))
                    # batched exp: merge adjacent written runs
                    pt = pt_pool.tile([128, GROUP * 2 * 256], MM2_DT, tag="pt")
                    merged = []
                    for rcol, rn in sorted(runs):
                        if merged and merged[-1][0] + merged[-1][1] == rcol:
                            merged[-1][1] += rn
                        else:
                            merged.append([rcol, rn])
                    for rcol, rn in merged:
                        nc.scalar.activation(pt[:, rcol:rcol + rn],
                                             ps[:, rcol:rcol + rn],
                                             Exp, scale=SCALE)
                    # MM2s + evacuation of completed windows
                    for s, p in enumerate(chunks):
                        for h in range(2):
                            col = h * (GROUP * 256) + s * 256
                            if p >= 1:
                                # block 0: window p-1 self-contribution (last)
                                wi = p - 1
                                t = out_ps[(h, wi)]
                                nc.tensor.matmul(
                                    t[:, 0:D + 1],
                                    pt[:, col:col + WS],
                                    vts[h][:, p, :],
                                    start=False, stop=True,
                                )
                                nc.vector.tensor_copy(unnorm[h][:, wi, :],
                                                      t[:, 0:D + 1])
                                del out_ps[(h, wi)]
                            if p <= w - 1:
                                # window p prev-contribution (first)
                                bcol = col + (WS if p >= 1 else 0)
                                t = ps_o.tile([128, D + 1], f32, tag="out")
                                out_ps[(h, p)] = t
                                nc.tensor.matmul(
                                    t[:, 0:D + 1],
                                    pt[:, bcol:bcol + WS],
                                    vts[h][:, p, :],
                                    start=True, stop=False,
                                )
                # per-head epilogue
                for h in range(2):
                    recip = rc_pool.tile([128, w], f32, tag="recip")
                    nc.vector.reciprocal(recip[:], unnorm[h][:, :, D])
                    stg = st_pool.tile([128, w, D], f32, tag="stg")
                    nc.vector.tensor_mul(
                        stg[:],
                        unnorm[h][:, :, 0:D],
                        recip[:, :, None].to_broadcast((128, w, D)),
                    )
                    nc.sync.dma_start(out[2 * pair + h], stg[:])

    nc.compile()
    return nc


def _get_nc():
    if "nc" not in _NC_CACHE:
        _NC_CACHE["nc"] = build_nc()
    return _NC_CACHE["nc"]


def _prep_core(qf, kf, vf, lo):
    """Build one core's input dict from flat [64, 4096, 64] fp32 arrays."""
    q8 = qf[lo:lo + HPC]                      # [8, 4096, 64]
    k8 = kf[lo:lo + HPC]
    v8 = vf[lo:lo + HPC]

    qT = np.ascontiguousarray(q8.transpose(0, 2, 1)).reshape(PAIRS, 128, N)
    qT = qT.astype(np.float16)

    pad = np.full((HPC, WS, D), -1.0, dtype=np.float32)
    kp = np.concatenate([pad, k8], axis=1)    # [8, 4224, 64]
    kT = np.ascontiguousarray(kp.transpose(0, 2, 1)).reshape(PAIRS, 128, C * WS)
    kT = kT.astype(np.float16)

    vp = np.concatenate([pad, v8], axis=1)    # [8, 4224, 64]
    ones = np.ones((HPC, C * WS, 1), dtype=np.float32)
    va = np.concatenate([vp, ones], axis=2)   # [8, 4224, 65]
    va = va.reshape(HPC, C, WS, D + 1).transpose(0, 2, 1, 3)  # [8, 128, 33, 65]
    va = np.ascontiguousarray(va).astype(np.float16)

    return {"qT": qT, "kT": kT, "v": va}


def kernel(q, k, v):
    q = np.asarray(q, dtype=np.float32)
    k = np.asarray(k, dtype=np.float32)
    v = np.asarray(v, dtype=np.float32)
    qf = q.reshape(B * H, N, D)
    kf = k.reshape(B * H, N, D)
    vf = v.reshape(B * H, N, D)

    nc = _get_nc()
    in_maps = [_prep_core(qf, kf, vf, HPC * c) for c in range(NC)]
    res = run_bass_kernel_spmd(nc, in_maps, core_ids=list(range(NC)))

    outs = []
    for c in range(NC):
        o = res.results[c]["out"]             # [8, 128, 32, 64]
        o = o.transpose(0, 2, 1, 3).reshape(HPC, N, D)
        outs.append(o)
    return np.concatenate(outs, axis=0).reshape(B, H, N, D).astype(np.float32)


if __name__ == "__main__":
    rng = np.random.default_rng(0)
    q = rng.standard_normal((B, H, N, D), dtype=np.float32)
    k = rng.standard_normal((B, H, N, D), dtype=np.float32)
    v = rng.standard_normal((B, H, N, D), dtype=np.float32)
    o = kernel(q, k, v)
    print("out", o.shape, o.dtype, float(np.abs(o).max()))
